# revision 2
# baseline (speedup 1.0000x reference)
# ConvLSTM block (B=4,T=16,H=W=64,Cin=32,Cout=64,K=3) + inference BatchNorm,
# as a Bass/Tile kernel for 8 trn2 NeuronCores.
#
# Sharding: core = b*2 + s  (b in 0..3 = batch sample, s in 0..1 = H-half).
# Each core owns 32 output rows of one sample and runs the full T=16 scan on a
# shrinking halo: at step t it computes h/c on (47-t) rows so that no
# inter-core communication is ever needed (the receptive field of the rows it
# owns at later steps is covered by redundantly computed halo rows).
# The s=1 half is vertically flipped on the host (data + kernel rows) so that
# both halves run the *same* SPMD program.
#
# Layout: channels on SBUF partitions, pixels on the free dim, rows padded to
# width 66 with zero columns (and one zero row above) so every 3x3 tap becomes
# a single flat pixel offset and SAME-padding comes out of reads of zeroed
# cells. Convs = per-tap matmuls (contract dim = input channels) accumulated
# in PSUM; the x-conv and h-conv for a step accumulate into the same PSUM
# tile. Gate order is permuted to chunk0=[f;i], chunk1=[o;cc] so that:
#   - hard_sigmoid(f,i) is one Relu-activation + one DVE min over 128 rows,
#   - tanh(cc) lands in the top half of the CT=[c;T] buffer,
#   - PP = [f*c ; i*T] is one 128-row DVE multiply,
#   - c' = PP_lo + PP_hi is a [I64;I64] selection matmul (cross-partition add),
#   - BatchNorm folds into the weights/gate-bias (y == scaled hidden state).
import math
from contextlib import ExitStack

import numpy as np

import concourse.bass as bass
import concourse.mybir as mybir
import concourse.tile as tile
from concourse import bass_utils

AF = mybir.ActivationFunctionType
ALU = mybir.AluOpType
F32 = mybir.dt.float32
F32R = mybir.dt.float32r

B, T, H, W = 4, 16, 64, 64
CIN, COUT = 32, 64
FR, FW = 49, 66          # frame rows / padded row width
NPIX = FR * FW           # 3234
NCORES = 8
PTILE = 512              # pixel tile (one PSUM bank of fp32)

# flat-pixel offset of conv tap (ky, kx), taps enumerated tau = ky*3 + kx
TAP_D = [(ky - 1) * FW + (kx - 1) for ky in range(3) for kx in range(3)]


def _build_nc(needs_affine: bool) -> bass.Bass:
    nc = bass.Bass(trn_type="TRN2", target_bir_lowering=False, debug=False)

    xin = nc.dram_tensor("xin", [T, CIN, FR, FW], F32, kind="ExternalInput").ap()
    wx_d = nc.dram_tensor("wx", [CIN, 9, 2, 128], F32, kind="ExternalInput").ap()
    wr_d = nc.dram_tensor("wrec", [COUT, 9, 2, 128], F32, kind="ExternalInput").ap()
    bv_d = nc.dram_tensor("bvec", [128, 2], F32, kind="ExternalInput").ap()
    iv_d = nc.dram_tensor("invv", [COUT, 3], F32, kind="ExternalInput").ap()
    sm_d = nc.dram_tensor("smat", [128, COUT], F32, kind="ExternalInput").ap()
    yout = nc.dram_tensor("yout", [T, COUT, 32, W], F32, kind="ExternalOutput").ap()

    with tile.TileContext(nc) as tc:
        with ExitStack() as ctx:
            consts = ctx.enter_context(tc.tile_pool(name="consts", bufs=1))
            xpool = ctx.enter_context(tc.tile_pool(name="xpool", bufs=2))
            gpool = ctx.enter_context(tc.tile_pool(name="gpool", bufs=2, space="PSUM"))
            cpool = ctx.enter_context(tc.tile_pool(name="cpool", bufs=2, space="PSUM"))

            wx = consts.tile([CIN, 9, 2, 128], F32, tag="wx")
            wr = consts.tile([COUT, 9, 2, 128], F32, tag="wr")
            bv = consts.tile([128, 2], F32, tag="bv")
            iv = consts.tile([COUT, 3], F32, tag="iv")
            sm = consts.tile([128, COUT], F32, tag="sm")
            nc.sync.dma_start(out=wx[:], in_=wx_d)
            nc.sync.dma_start(out=wr[:], in_=wr_d)
            nc.sync.dma_start(out=bv[:], in_=bv_d)
            nc.sync.dma_start(out=iv[:], in_=iv_d)
            nc.sync.dma_start(out=sm[:], in_=sm_d)

            # persistent state / scratch buffers
            hb = [consts.tile([COUT, NPIX], F32, tag=f"h{k}", name=f"h{k}") for k in range(2)]
            CT = consts.tile([128, NPIX], F32, tag="ct")    # [c ; tanh(cc)]
            IFs = consts.tile([128, NPIX], F32, tag="ifs")  # [f_s ; i_s]
            PP = consts.tile([128, NPIX], F32, tag="pp")    # [f*c ; i*T]
            ost = consts.tile([COUT, NPIX], F32, tag="ost")  # o_s (* inv)
            thc = consts.tile([COUT, NPIX], F32, tag="thc")  # tanh(c')
            yst = consts.tile([COUT, NPIX], F32, tag="yst", name="yst") if needs_affine else None

            nc.vector.memset(hb[0][:], 0.0)
            nc.vector.memset(hb[1][:], 0.0)
            nc.vector.memset(CT[:], 0.0)

            for t in range(T):
                rows = 47 - t                      # computed rows this step
                xt = xpool.tile([CIN, NPIX], F32, tag="xt", name="xt")
                nx = (rows + 2) * FW               # x pixels needed (rows +/- 1)
                nc.sync.dma_start(
                    out=xt[:, 0:nx],
                    in_=xin[t].rearrange("c r w -> c (r w)")[:, 0:nx],
                )
                h_prev = hb[(t + 1) % 2]
                h_cur = hb[t % 2]

                p_lo = FW + 1                      # first real pixel (row 1, col 1)
                cnt = rows * FW - 2                # through last real pixel
                ntl = math.ceil(cnt / PTILE)
                for j in range(ntl):
                    pj = p_lo + j * PTILE
                    nt = min(PTILE, p_lo + cnt - pj)
                    g0 = gpool.tile([128, PTILE], F32, tag="g0", name="g0")
                    g1 = gpool.tile([128, PTILE], F32, tag="g1", name="g1")
                    for m, g in ((0, g0), (1, g1)):
                        mms = [
                            (wx[:, tau, m, :], xt[:, pj + d: pj + d + nt])
                            for tau, d in enumerate(TAP_D)
                        ]
                        if t > 0:
                            mms += [
                                (wr[:, tau, m, :], h_prev[:, pj + d: pj + d + nt])
                                for tau, d in enumerate(TAP_D)
                            ]
                        for k, (lh, rh) in enumerate(mms):
                            nc.tensor.matmul(
                                g[:, 0:nt],
                                lh.bitcast(F32R),
                                rh.bitcast(F32R),
                                start=(k == 0),
                                stop=(k == len(mms) - 1),
                            )
                    sl = slice(pj, pj + nt)
                    # hard_sigmoid pre-clip for [f;i] and o; tanh for cc
                    nc.scalar.activation(
                        IFs[:, sl], g0[:, 0:nt], AF.Relu, bias=bv[:, 0:1], scale=0.2
                    )
                    nc.scalar.activation(
                        ost[:, sl], g1[0:COUT, 0:nt], AF.Relu,
                        bias=bv[0:COUT, 1:2], scale=0.2,
                    )
                    nc.scalar.activation(
                        CT[COUT:128, sl], g1[COUT:128, 0:nt], AF.Tanh,
                        bias=bv[COUT:128, 1:2],
                    )
                    nc.vector.tensor_scalar(
                        IFs[:, sl], IFs[:, sl], 1.0, None, op0=ALU.min
                    )
                    nc.vector.tensor_scalar(
                        ost[:, sl], ost[:, sl], 1.0, iv[:, 0:1],
                        op0=ALU.min, op1=ALU.mult,
                    )
                    nc.vector.tensor_mul(PP[:, sl], IFs[:, sl], CT[:, sl])
                    # c' = f*c + i*T  via selection matmul (exact fp32)
                    cp = cpool.tile([COUT, PTILE], F32, tag="cp", name="cp")
                    nc.tensor.matmul(
                        cp[:, 0:nt], sm[:], PP[:, sl], start=True, stop=True
                    )
                    nc.scalar.activation(CT[0:COUT, sl], cp[:, 0:nt], AF.Copy)
                    nc.scalar.activation(thc[:, sl], cp[:, 0:nt], AF.Tanh)

                # h = o_s * tanh(c'), written only to real (non-pad) cells
                def _v(buf):
                    return buf[:].rearrange("p (r w) -> p r w", w=FW)[
                        :, 1: rows + 1, 1: W + 1
                    ]

                nc.vector.tensor_mul(_v(h_cur), _v(ost), _v(thc))
                if needs_affine:
                    nc.scalar.activation(
                        _v(yst), _v(h_cur), AF.Identity,
                        bias=iv[:, 2:3], scale=iv[:, 1:2],
                    )
                ysrc = yst if needs_affine else h_cur
                nc.sync.dma_start(
                    out=yout[t],
                    in_=ysrc[:].rearrange("p (r w) -> p r w", w=FW)[:, 1:33, 1: W + 1],
                )
    return nc


def prepare(x, kernel, rec_kernel, bias, gamma, beta, moving_mean, moving_var):
    """Host-side prep: BN folding, gate permutation, per-core shards."""
    x = np.asarray(x, np.float32)
    kernel = np.asarray(kernel, np.float32)
    rec_kernel = np.asarray(rec_kernel, np.float32)
    bias = np.asarray(bias, np.float32)
    inv = np.asarray(gamma, np.float32) / np.sqrt(
        np.asarray(moving_var, np.float32) + 1e-3
    )
    shift = np.asarray(beta, np.float32) - np.asarray(moving_mean, np.float32) * inv
    fold = bool(np.max(np.abs(shift)) == 0.0)

    # gate blocks in reference order: i,f,cc,o -> chunk0=[f;i], chunk1=[o;cc]
    perm = np.concatenate(
        [np.arange(64, 128), np.arange(0, 64), np.arange(192, 256), np.arange(128, 192)]
    )
    wx_e = kernel[:, :, :, perm]
    wr_e = rec_kernel[:, :, :, perm]
    if fold:
        # state becomes h' = h * inv  (== BN output y); compensate h-conv input
        wr_e = wr_e / inv[None, None, :, None]
    b_p = bias[perm]
    bv0 = 0.5 + 0.2 * b_p[0:128]
    bv1 = np.concatenate([0.5 + 0.2 * b_p[128:192], b_p[192:256]])
    bvec = np.stack([bv0, bv1], axis=1).astype(np.float32)
    ivv = np.stack(
        [inv if fold else np.ones(COUT, np.float32), inv, shift], axis=1
    ).astype(np.float32)
    smat = np.concatenate([np.eye(COUT), np.eye(COUT)], axis=0).astype(np.float32)

    def wpack(w):  # [3,3,cin,256] -> [cin, 9, 2, 128]
        cin = w.shape[2]
        return np.ascontiguousarray(
            w.reshape(9, cin, 2, 128).transpose(1, 0, 2, 3)
        ).astype(np.float32)

    in_maps = []
    for core in range(NCORES):
        b, s = core // 2, core % 2
        xs = x[b] if s == 0 else x[b, :, ::-1]
        wx_s = wx_e if s == 0 else wx_e[::-1]
        wr_s = wr_e if s == 0 else wr_e[::-1]
        xf = np.zeros((T, CIN, FR, FW), np.float32)
        xf[:, :, 1:49, 1: W + 1] = xs[:, 0:48].transpose(0, 3, 1, 2)
        in_maps.append(
            dict(
                xin=xf,
                wx=wpack(wx_s),
                wrec=wpack(wr_s),
                bvec=bvec,
                invv=ivv,
                smat=smat,
            )
        )
    return in_maps, fold


def assemble(results):
    y = np.zeros((B, T, H, W, COUT), np.float32)
    for core in range(NCORES):
        b, s = core // 2, core % 2
        blk = results[core]["yout"].transpose(0, 2, 3, 1)  # [T, 32, W, C]
        if s == 0:
            y[b, :, 0:32] = blk
        else:
            y[b, :, 32:64] = blk[:, ::-1]
    return y


_NC_CACHE: dict = {}


def get_nc(needs_affine: bool) -> bass.Bass:
    if needs_affine not in _NC_CACHE:
        _NC_CACHE[needs_affine] = _build_nc(needs_affine)
    return _NC_CACHE[needs_affine]


def kernel(**inputs) -> np.ndarray:
    in_maps, fold = prepare(**inputs)
    nc = get_nc(not fold)
    res = bass_utils.run_bass_kernel_spmd(nc, in_maps, core_ids=list(range(NCORES)))
    return assemble(res.results)


# revision 8
# speedup vs baseline: 1.3297x; 1.3297x over previous
# ConvLSTM block (B=4,T=16,H=W=64,Cin=32,Cout=64,K=3) + inference BatchNorm,
# as a Bass/Tile kernel for 8 trn2 NeuronCores.
#
# Sharding: core = b*2 + s  (b in 0..3 = batch sample, s in 0..1 = H-half).
# Each core owns 32 output rows of one sample and runs the full T=16 scan on a
# shrinking halo: at step t it computes h/c on (47-t) rows so that no
# inter-core communication is ever needed. The s=1 half is vertically flipped
# on the host (data + kernel rows) so both halves run the same SPMD program.
#
# Layout: channels on SBUF partitions, pixels on the free dim, rows padded to
# width 66 with zero columns (and one zero row above) so every 3x3 tap becomes
# a single flat pixel offset and SAME-padding comes out of reads of zeroed
# cells.
#
# Conv = matmuls with contract dim = (tap, channel) packed to K=128 by
# PHYSICALLY stacking shifted copies of the image in the partition dim:
#   x4  = [x@0; x@+1; x@+66; x@+67]  -> 2 quad-tap matmuls (one tap zeroed dup)
#   x2b = [x@0; x@+130]              -> 1 dual-tap matmul (K=64)
#   h2a = [h@0; h@+66]               -> 3 dual-tap matmuls + 1 solo (K=64)
#   h2b = [h@0; h@+1]                -> 1 dual-tap matmul
# so a step costs 8 conv matmuls per output chunk instead of 18. Shifts are
# baked in at DMA time (x: 6 HBM loads, h: 3 SBUF-SBUF dup DMAs per step).
#
# Gate order is permuted to chunk0=[f;i], chunk1=[o;cc] so that:
#   - hard_sigmoid(f,i) is one Relu-activation + one DVE min over 128 rows,
#   - tanh(cc) lands in the top half of the CT=[c;T] buffer,
#   - PP = [f*c ; i*T] is one 128-row DVE multiply,
#   - c' = PP_lo + PP_hi is a [I64;I64] selection matmul (cross-partition add),
#   - BatchNorm folds into the weights/gate-bias (y == scaled hidden state).
import math
from contextlib import ExitStack

import numpy as np

import concourse.bacc as bacc
import concourse.bass as bass
import concourse.mybir as mybir
import concourse.tile as tile
from concourse import bass_utils

AF = mybir.ActivationFunctionType
ALU = mybir.AluOpType
F32 = mybir.dt.float32
F16 = mybir.dt.float16

B, T, H, W = 4, 16, 64, 64
CIN, COUT = 32, 64
FR, FW = 49, 66          # frame rows / padded row width
NPIX = FR * FW           # 3234
NCORES = 8
PTILE = 512              # pixel tile (one PSUM bank of fp32)

# (slot, K, rhs base offset, rhs buffer) for the stacked conv matmuls
XSLOTS = [(0, 128, -67, "x4"), (1, 128, 0, "x4"), (2, 64, -65, "xb")]
HSLOTS = [(0, 128, -67, "a"), (1, 128, -66, "a"), (2, 128, -65, "a"),
          (3, 128, 65, "b"), (4, 64, 67, "a")]
X4_SHIFTS = (0, 1, 66, 67)
XB_SHIFTS = (0, 130)


def _build_nc(needs_affine: bool) -> bass.Bass:
    nc = bacc.Bacc("TRN2", target_bir_lowering=False, debug=False)

    xin = nc.dram_tensor("xin", [T, CIN, FR, FW], F16, kind="ExternalInput").ap()
    wxs_d = nc.dram_tensor("wxs", [128, 3, 2, 128], F16, kind="ExternalInput").ap()
    wrs_d = nc.dram_tensor("wrs", [128, 5, 2, 128], F16, kind="ExternalInput").ap()
    bv_d = nc.dram_tensor("bvec", [128, 2], F32, kind="ExternalInput").ap()
    iv_d = nc.dram_tensor("invv", [COUT, 3], F32, kind="ExternalInput").ap()
    sm_d = nc.dram_tensor("smat", [128, COUT], F16, kind="ExternalInput").ap()
    yout = nc.dram_tensor("yout", [T, COUT, 32, W], F32, kind="ExternalOutput").ap()

    with tile.TileContext(nc) as tc:
        with ExitStack() as ctx:
            consts = ctx.enter_context(tc.tile_pool(name="consts", bufs=1))
            xpool = ctx.enter_context(tc.tile_pool(name="xpool", bufs=2))
            gpool = ctx.enter_context(tc.tile_pool(name="gpool", bufs=2, space="PSUM"))
            cpool = ctx.enter_context(tc.tile_pool(name="cpool", bufs=2, space="PSUM"))

            wxs = consts.tile([128, 3, 2, 128], F16, tag="wxs")
            wrs = consts.tile([128, 5, 2, 128], F16, tag="wrs")
            bv = consts.tile([128, 2], F32, tag="bv")
            iv = consts.tile([COUT, 3], F32, tag="iv")
            sm = consts.tile([128, COUT], F16, tag="sm")
            nc.sync.dma_start(out=wxs[:], in_=wxs_d)
            nc.sync.dma_start(out=wrs[:], in_=wrs_d)
            nc.sync.dma_start(out=bv[:], in_=bv_d)
            nc.sync.dma_start(out=iv[:], in_=iv_d)
            nc.sync.dma_start(out=sm[:], in_=sm_d)

            # persistent state / scratch buffers
            h2a = [consts.tile([128, NPIX], F16, tag=f"h2a{k}", name=f"h2a{k}")
                   for k in range(2)]
            h2b = [consts.tile([128, NPIX], F16, tag=f"h2b{k}", name=f"h2b{k}")
                   for k in range(2)]
            CT = consts.tile([128, NPIX], F32, tag="ct")    # [c ; tanh(cc)]
            IFs = consts.tile([128, NPIX], F32, tag="ifs")  # [f_s ; i_s]
            PP = consts.tile([128, NPIX], F16, tag="pp")    # [f*c ; i*T]
            ost = consts.tile([COUT, NPIX], F32, tag="ost")  # o_s (* inv)
            thc = consts.tile([COUT, NPIX], F32, tag="thc")  # tanh(c')
            yst = (consts.tile([COUT, NPIX], F32, tag="yst", name="yst")
                   if needs_affine else None)

            for k in range(2):
                nc.vector.memset(h2a[k][:], 0.0)
                nc.vector.memset(h2b[k][:], 0.0)
            nc.vector.memset(CT[:], 0.0)

            for t in range(T):
                rows = 47 - t                      # computed rows this step
                nx = (rows + 2) * FW               # x pixels needed (rows +/- 1)
                xf = xin[t].rearrange("c r w -> c (r w)")
                x4 = xpool.tile([128, NPIX], F16, tag="x4", name="x4")
                xb = xpool.tile([64, NPIX], F16, tag="xb", name="xb")
                for k, s in enumerate(X4_SHIFTS):
                    nc.sync.dma_start(out=x4[32 * k:32 * (k + 1), 0:nx - s],
                                      in_=xf[:, s:nx])
                for k, s in enumerate(XB_SHIFTS):
                    nc.sync.dma_start(out=xb[32 * k:32 * (k + 1), 0:nx - s],
                                      in_=xf[:, s:nx])
                ha_prev, ha_cur = h2a[(t + 1) % 2], h2a[t % 2]
                hb_prev = h2b[(t + 1) % 2]
                hb_cur = h2b[t % 2]

                p_lo = FW + 1                      # first real pixel (row 1, col 1)
                cnt = rows * FW - 2                # through last real pixel
                ntl = math.ceil(cnt / PTILE)
                for j in range(ntl):
                    pj = p_lo + j * PTILE
                    nt = min(PTILE, p_lo + cnt - pj)
                    g0 = gpool.tile([128, PTILE], F32, tag="g0", name="g0")
                    g1 = gpool.tile([128, PTILE], F32, tag="g1", name="g1")
                    for m, g in ((0, g0), (1, g1)):
                        mms = [
                            (wxs[0:K, slot, m, :],
                             (x4 if bufn == "x4" else xb)[0:K, pj + d: pj + d + nt])
                            for slot, K, d, bufn in XSLOTS
                        ]
                        if t > 0:
                            mms += [
                                (wrs[0:K, slot, m, :],
                                 (ha_prev if bufn == "a" else hb_prev)[
                                     0:K, pj + d: pj + d + nt])
                                for slot, K, d, bufn in HSLOTS
                            ]
                        for k, (lh, rh) in enumerate(mms):
                            nc.tensor.matmul(
                                g[:, 0:nt], lh, rh,
                                start=(k == 0), stop=(k == len(mms) - 1),
                            )
                    sl = slice(pj, pj + nt)
                    # hard_sigmoid pre-clip for [f;i] and o; tanh for cc
                    nc.scalar.activation(
                        IFs[:, sl], g0[:, 0:nt], AF.Relu, bias=bv[:, 0:1], scale=0.2
                    )
                    nc.scalar.activation(
                        ost[:, sl], g1[0:COUT, 0:nt], AF.Relu,
                        bias=bv[0:COUT, 1:2], scale=0.2,
                    )
                    nc.scalar.activation(
                        CT[COUT:128, sl], g1[COUT:128, 0:nt], AF.Tanh,
                        bias=bv[COUT:128, 1:2],
                    )
                    nc.vector.tensor_scalar(
                        IFs[:, sl], IFs[:, sl], 1.0, None, op0=ALU.min
                    )
                    nc.gpsimd.tensor_scalar(
                        ost[:, sl], ost[:, sl], 1.0, iv[:, 0:1],
                        op0=ALU.min, op1=ALU.mult,
                    )
                    nc.vector.tensor_mul(PP[:, sl], IFs[:, sl], CT[:, sl])
                    # c' = f*c + i*T  via selection matmul
                    cp = cpool.tile([COUT, PTILE], F32, tag="cp", name="cp")
                    nc.tensor.matmul(
                        cp[:, 0:nt], sm[:], PP[:, sl], start=True, stop=True
                    )
                    nc.scalar.activation(CT[0:COUT, sl], cp[:, 0:nt], AF.Copy)
                    nc.scalar.activation(thc[:, sl], cp[:, 0:nt], AF.Tanh)

                # h = o_s * tanh(c'), written only to real (non-pad) cells
                def _v(buf):
                    return buf.rearrange("p (r w) -> p r w", w=FW)[
                        :, 1: rows + 1, 1: W + 1
                    ]

                nc.vector.tensor_mul(_v(ha_cur[0:COUT, :]), _v(ost[:]), _v(thc[:]))
                # shifted duplicates for next step's packed h matmuls
                L = (rows + 1) * FW
                nc.sync.dma_start(out=ha_cur[64:128, 0:L - 66],
                                  in_=ha_cur[0:64, 66:L])
                nc.sync.dma_start(out=hb_cur[0:64, 0:L], in_=ha_cur[0:64, 0:L])
                nc.sync.dma_start(out=hb_cur[64:128, 0:L - 1],
                                  in_=ha_cur[0:64, 1:L])
                if needs_affine:
                    nc.scalar.activation(
                        _v(yst[:]), _v(ha_cur[0:COUT, :]), AF.Identity,
                        bias=iv[:, 2:3], scale=iv[:, 1:2],
                    )
                    nc.sync.dma_start(
                        out=yout[t],
                        in_=yst[:].rearrange("p (r w) -> p r w", w=FW)[
                            :, 1:33, 1: W + 1],
                    )
                else:
                    nc.gpsimd.dma_start(
                        out=yout[t],
                        in_=ha_cur[0:COUT, :].rearrange("p (r w) -> p r w", w=FW)[
                            :, 1:33, 1: W + 1],
                    )
    nc.compile()
    return nc


def prepare(x, kernel, rec_kernel, bias, gamma, beta, moving_mean, moving_var):
    """Host-side prep: BN folding, gate permutation, per-core shards."""
    x = np.asarray(x, np.float32)
    kernel = np.asarray(kernel, np.float32)
    rec_kernel = np.asarray(rec_kernel, np.float32)
    bias = np.asarray(bias, np.float32)
    inv = np.asarray(gamma, np.float32) / np.sqrt(
        np.asarray(moving_var, np.float32) + 1e-3
    )
    shift = np.asarray(beta, np.float32) - np.asarray(moving_mean, np.float32) * inv
    fold = bool(np.max(np.abs(shift)) == 0.0)

    # gate blocks in reference order: i,f,cc,o -> chunk0=[f;i], chunk1=[o;cc]
    perm = np.concatenate(
        [np.arange(64, 128), np.arange(0, 64), np.arange(192, 256), np.arange(128, 192)]
    )
    wx_e = kernel[:, :, :, perm]
    wr_e = rec_kernel[:, :, :, perm]
    if fold:
        # state becomes h' = h * inv  (== BN output y); compensate h-conv input
        wr_e = wr_e / inv[None, None, :, None]
    b_p = bias[perm]
    bv0 = 0.5 + 0.2 * b_p[0:128]
    bv1 = np.concatenate([0.5 + 0.2 * b_p[128:192], b_p[192:256]])
    bvec = np.stack([bv0, bv1], axis=1).astype(np.float32)
    ivv = np.stack(
        [inv if fold else np.ones(COUT, np.float32), inv, shift], axis=1
    ).astype(np.float32)
    smat = np.concatenate([np.eye(COUT), np.eye(COUT)], axis=0).astype(np.float16)

    def stack_x(w):  # [3,3,32,256] -> [128, 3, 2, 128] slot stacks
        S = np.zeros((128, 3, 2, 128), np.float32)

        def put(slot, band, ky, kx):
            blk = w[ky, kx]  # [32, 256]
            for m in (0, 1):
                S[band * 32:(band + 1) * 32, slot, m, :] = blk[:, m * 128:(m + 1) * 128]

        put(0, 0, 0, 0); put(0, 1, 0, 1); put(0, 2, 1, 0); put(0, 3, 1, 1)
        put(1, 1, 1, 2); put(1, 2, 2, 1); put(1, 3, 2, 2)   # band 0 = zeroed dup
        put(2, 0, 0, 2); put(2, 1, 2, 0)
        return S.astype(np.float16)

    def stack_h(w):  # [3,3,64,256] -> [128, 5, 2, 128] slot stacks
        S = np.zeros((128, 5, 2, 128), np.float32)

        def put(slot, half, ky, kx):
            blk = w[ky, kx]  # [64, 256]
            for m in (0, 1):
                S[half * 64:(half + 1) * 64, slot, m, :] = blk[:, m * 128:(m + 1) * 128]

        for kx in range(3):
            put(kx, 0, 0, kx); put(kx, 1, 1, kx)
        put(3, 0, 2, 0); put(3, 1, 2, 1)
        put(4, 0, 2, 2)
        return S.astype(np.float16)

    in_maps = []
    for core in range(NCORES):
        b, s = core // 2, core % 2
        xs = x[b] if s == 0 else x[b, :, ::-1]
        wx_s = wx_e if s == 0 else wx_e[::-1]
        wr_s = wr_e if s == 0 else wr_e[::-1]
        xf = np.zeros((T, CIN, FR, FW), np.float16)
        xf[:, :, 1:49, 1: W + 1] = xs[:, 0:48].transpose(0, 3, 1, 2)
        in_maps.append(
            dict(
                xin=xf,
                wxs=stack_x(wx_s),
                wrs=stack_h(wr_s),
                bvec=bvec,
                invv=ivv,
                smat=smat,
            )
        )
    return in_maps, fold


def assemble(results):
    y = np.zeros((B, T, H, W, COUT), np.float32)
    for core in range(NCORES):
        b, s = core // 2, core % 2
        blk = results[core]["yout"].transpose(0, 2, 3, 1)  # [T, 32, W, C]
        if s == 0:
            y[b, :, 0:32] = blk
        else:
            y[b, :, 32:64] = blk[:, ::-1]
    return y


_NC_CACHE: dict = {}


def get_nc(needs_affine: bool) -> bass.Bass:
    if needs_affine not in _NC_CACHE:
        _NC_CACHE[needs_affine] = _build_nc(needs_affine)
    return _NC_CACHE[needs_affine]


def kernel(**inputs) -> np.ndarray:
    in_maps, fold = prepare(**inputs)
    nc = get_nc(not fold)
    res = bass_utils.run_bass_kernel_spmd(nc, in_maps, core_ids=list(range(NCORES)))
    return assemble(res.results)


# revision 11
# speedup vs baseline: 2.1920x; 1.6485x over previous
# ConvLSTM block (B=4,T=16,H=W=64,Cin=32,Cout=64,K=3) + inference BatchNorm,
# as a Bass/Tile kernel for 8 trn2 NeuronCores.
#
# Sharding: core = b*2 + s  (b in 0..3 = batch sample, s in 0..1 = H-half).
# Each core owns 32 output rows of one sample and runs the full T=16 scan on a
# shrinking halo: at step t it computes h/c on (47-t) rows so that no
# inter-core communication is ever needed. The s=1 half is vertically flipped
# on the host (data + kernel rows) so both halves run the same SPMD program.
#
# Layout: channels on SBUF partitions, pixels on the free dim, rows padded to
# width 66 with zero columns (and one zero row above) so every 3x3 tap becomes
# a single flat pixel offset and SAME-padding comes out of reads of zeroed
# cells.
#
# Conv = matmuls with contract dim = (tap, channel) packed to K=128 by
# PHYSICALLY stacking shifted copies of the image in the partition dim:
#   x4  = [x@0; x@+1; x@+66; x@+67]  -> 2 quad-tap matmuls (one tap zeroed dup)
#   x2b = [x@0; x@+130]              -> 1 dual-tap matmul (K=64)
#   h2a = [h@0; h@+66]               -> 3 dual-tap matmuls + 1 solo (K=64)
#   h2b = [h@0; h@+1]                -> 1 dual-tap matmul
# so a step costs 8 conv matmuls per output chunk instead of 18. Shifts are
# baked in at DMA time (x: 6 HBM loads, h: 3 SBUF-SBUF dup DMAs per step).
#
# Gate order is permuted to chunk0=[f;i], chunk1=[o;cc] so that:
#   - hard_sigmoid(f,i) is one Relu-activation + one DVE min over 128 rows,
#   - tanh(cc) lands in the top half of the CT=[c;T] buffer,
#   - PP = [f*c ; i*T] is one 128-row DVE multiply,
#   - c' = PP_lo + PP_hi is a [I64;I64] selection matmul (cross-partition add),
#   - BatchNorm folds into the weights/gate-bias (y == scaled hidden state).
import math
from contextlib import ExitStack

import numpy as np

import concourse.bacc as bacc
import concourse.bass as bass
import concourse.mybir as mybir
import concourse.tile as tile
from concourse import bass_utils

AF = mybir.ActivationFunctionType
ALU = mybir.AluOpType
F32 = mybir.dt.float32
F16 = mybir.dt.float16

B, T, H, W = 4, 16, 64, 64
CIN, COUT = 32, 64
FR, FW = 49, 66          # frame rows / padded row width
NPIX = FR * FW           # 3234
NCORES = 8
PTILE = 512              # pixel tile (one PSUM bank of fp32)

# (slot, K, rhs base offset, rhs buffer) for the stacked conv matmuls
XSLOTS = [(0, 128, -67, "x4"), (1, 128, 0, "x4"), (2, 64, -65, "xb")]
HSLOTS = [(0, 128, -67, "a"), (1, 128, -66, "a"), (2, 128, -65, "a"),
          (3, 128, 65, "b"), (4, 64, 67, "a")]
X4_SHIFTS = (0, 1, 66, 67)
XB_SHIFTS = (0, 130)


def _build_nc(needs_affine: bool) -> bass.Bass:
    nc = bacc.Bacc("TRN2", target_bir_lowering=False, debug=False)

    xin = nc.dram_tensor("xin", [T, CIN, FR, FW], F16, kind="ExternalInput").ap()
    wxs_d = nc.dram_tensor("wxs", [128, 3, 2, 128], F16, kind="ExternalInput").ap()
    wrs_d = nc.dram_tensor("wrs", [128, 5, 2, 128], F16, kind="ExternalInput").ap()
    bv_d = nc.dram_tensor("bvec", [128, 2], F32, kind="ExternalInput").ap()
    iv_d = nc.dram_tensor("invv", [COUT, 3], F32, kind="ExternalInput").ap()
    sm_d = nc.dram_tensor("smat", [128, COUT], F16, kind="ExternalInput").ap()
    yout = nc.dram_tensor("yout", [T, COUT, 32, W], F32, kind="ExternalOutput").ap()

    with tile.TileContext(nc) as tc:
        with ExitStack() as ctx:
            consts = ctx.enter_context(tc.tile_pool(name="consts", bufs=1))
            xpool = ctx.enter_context(tc.tile_pool(name="xpool", bufs=3))
            gpool = ctx.enter_context(tc.tile_pool(name="gpool", bufs=3, space="PSUM"))
            cpool = ctx.enter_context(tc.tile_pool(name="cpool", bufs=2, space="PSUM"))

            wxs = consts.tile([128, 3, 2, 128], F16, tag="wxs")
            wrs = consts.tile([128, 5, 2, 128], F16, tag="wrs")
            bv = consts.tile([128, 2], F32, tag="bv")
            iv = consts.tile([COUT, 3], F32, tag="iv")
            sm = consts.tile([128, COUT], F16, tag="sm")
            nc.sync.dma_start(out=wxs[:], in_=wxs_d)
            nc.sync.dma_start(out=wrs[:], in_=wrs_d)
            nc.sync.dma_start(out=bv[:], in_=bv_d)
            nc.sync.dma_start(out=iv[:], in_=iv_d)
            nc.sync.dma_start(out=sm[:], in_=sm_d)

            # persistent state / scratch buffers
            h2a = [consts.tile([128, NPIX], F16, tag=f"h2a{k}", name=f"h2a{k}")
                   for k in range(2)]
            h2b = [consts.tile([128, NPIX], F16, tag=f"h2b{k}", name=f"h2b{k}")
                   for k in range(2)]
            CT = consts.tile([128, NPIX], F32, tag="ct")    # [c ; tanh(cc)]
            IFs = consts.tile([128, NPIX], F32, tag="ifs")  # [f_s ; i_s]
            PP = consts.tile([128, NPIX], F16, tag="pp")    # [f*c ; i*T]
            ost = consts.tile([COUT, NPIX], F32, tag="ost")  # o_s (* inv)
            thc = consts.tile([COUT, NPIX], F32, tag="thc")  # tanh(c')
            yst = (consts.tile([COUT, NPIX], F32, tag="yst", name="yst")
                   if needs_affine else None)

            for k in range(2):
                nc.vector.memset(h2a[k][:], 0.0)
                nc.vector.memset(h2b[k][:], 0.0)
            nc.vector.memset(CT[:], 0.0)

            for t in range(T):
                rows = 47 - t                      # computed rows this step
                nx = (rows + 2) * FW               # x pixels needed (rows +/- 1)
                xf = xin[t].rearrange("c r w -> c (r w)")
                x4 = xpool.tile([128, NPIX], F16, tag="x4", name="x4")
                xb = xpool.tile([64, NPIX], F16, tag="xb", name="xb")
                for k, s in enumerate(X4_SHIFTS):
                    nc.sync.dma_start(out=x4[32 * k:32 * (k + 1), 0:nx - s],
                                      in_=xf[:, s:nx])
                for k, s in enumerate(XB_SHIFTS):
                    nc.sync.dma_start(out=xb[32 * k:32 * (k + 1), 0:nx - s],
                                      in_=xf[:, s:nx])
                ha_prev, ha_cur = h2a[(t + 1) % 2], h2a[t % 2]
                hb_prev = h2b[(t + 1) % 2]
                hb_cur = h2b[t % 2]

                p_lo = FW + 1                      # first real pixel (row 1, col 1)
                cnt = rows * FW - 2                # through last real pixel
                ntl = math.ceil(cnt / PTILE)
                bsz = math.ceil(cnt / ntl)
                for j in range(ntl):
                    pj = p_lo + j * bsz
                    nt = min(bsz, p_lo + cnt - pj)
                    g0 = gpool.tile([128, PTILE], F32, tag="g0", name="g0")
                    g1 = gpool.tile([128, PTILE], F32, tag="g1", name="g1")
                    for m, g in ((0, g0), (1, g1)):
                        mms = [
                            (wxs[0:K, slot, m, :],
                             (x4 if bufn == "x4" else xb)[0:K, pj + d: pj + d + nt])
                            for slot, K, d, bufn in XSLOTS
                        ]
                        if t > 0:
                            mms += [
                                (wrs[0:K, slot, m, :],
                                 (ha_prev if bufn == "a" else hb_prev)[
                                     0:K, pj + d: pj + d + nt])
                                for slot, K, d, bufn in HSLOTS
                            ]
                        for k, (lh, rh) in enumerate(mms):
                            nc.tensor.matmul(
                                g[:, 0:nt], lh, rh,
                                start=(k == 0), stop=(k == len(mms) - 1),
                            )
                    sl = slice(pj, pj + nt)
                    # hard_sigmoid pre-clip for [f;i] and o; tanh for cc
                    nc.scalar.activation(
                        IFs[:, sl], g0[:, 0:nt], AF.Relu, bias=bv[:, 0:1], scale=0.2
                    )
                    nc.scalar.activation(
                        ost[:, sl], g1[0:COUT, 0:nt], AF.Relu,
                        bias=bv[0:COUT, 1:2], scale=0.2,
                    )
                    nc.scalar.activation(
                        CT[COUT:128, sl], g1[COUT:128, 0:nt], AF.Tanh,
                        bias=bv[COUT:128, 1:2],
                    )
                    nc.vector.tensor_scalar(
                        IFs[:, sl], IFs[:, sl], 1.0, None, op0=ALU.min
                    )
                    nc.gpsimd.tensor_scalar(
                        ost[:, sl], ost[:, sl], 1.0, iv[:, 0:1],
                        op0=ALU.min, op1=ALU.mult,
                    )
                    nc.vector.tensor_mul(PP[:, sl], IFs[:, sl], CT[:, sl])
                    # c' = f*c + i*T  via selection matmul
                    cp = cpool.tile([COUT, PTILE], F32, tag="cp", name="cp")
                    nc.tensor.matmul(
                        cp[:, 0:nt], sm[:], PP[:, sl], start=True, stop=True
                    )
                    nc.scalar.activation(CT[0:COUT, sl], cp[:, 0:nt], AF.Copy)
                    nc.scalar.activation(thc[:, sl], cp[:, 0:nt], AF.Tanh)

                # h = o_s * tanh(c') in two row-halves so the step tail
                # (h-mult -> dup DMAs -> next step's h matmuls) pipelines with
                # the last tiles' convs instead of serializing behind them.
                def _vr(buf, r0, r1):
                    return buf.rearrange("p (r w) -> p r w", w=FW)[
                        :, r0:r1, 1: W + 1
                    ]

                mid = (rows + 1) // 2
                L = (rows + 1) * FW
                M = (1 + mid) * FW
                for r0, r1, a, b in ((1, 1 + mid, FW, M), (1 + mid, rows + 1, M, L)):
                    nc.vector.tensor_mul(
                        _vr(ha_cur[0:COUT, :], r0, r1),
                        _vr(ost[:], r0, r1), _vr(thc[:], r0, r1),
                    )
                    # shifted duplicates for next step's packed h matmuls
                    first = a == FW
                    nc.sync.dma_start(out=ha_cur[64:128, a - 66:b - 66],
                                      in_=ha_cur[0:64, a:b])
                    lo = 0 if first else a
                    nc.sync.dma_start(out=hb_cur[0:64, lo:b],
                                      in_=ha_cur[0:64, lo:b])
                    s1 = 1 if first else a
                    nc.sync.dma_start(out=hb_cur[64:128, s1 - 1:b - 1],
                                      in_=ha_cur[0:64, s1:b])
                    if needs_affine:
                        nc.scalar.activation(
                            _vr(yst[:], r0, r1), _vr(ha_cur[0:COUT, :], r0, r1),
                            AF.Identity, bias=iv[:, 2:3], scale=iv[:, 1:2],
                        )
                    ys = yst if needs_affine else ha_cur
                    ydma = nc.sync.dma_start if needs_affine else nc.gpsimd.dma_start
                    y0, y1 = r0, min(33, r1)
                    if y0 < 33:
                        ydma(
                            out=yout[t, :, y0 - 1:y1 - 1, :],
                            in_=ys[0:COUT, :].rearrange("p (r w) -> p r w", w=FW)[
                                :, y0:y1, 1: W + 1],
                        )
    nc.compile()
    return nc


def prepare(x, kernel, rec_kernel, bias, gamma, beta, moving_mean, moving_var):
    """Host-side prep: BN folding, gate permutation, per-core shards."""
    x = np.asarray(x, np.float32)
    kernel = np.asarray(kernel, np.float32)
    rec_kernel = np.asarray(rec_kernel, np.float32)
    bias = np.asarray(bias, np.float32)
    inv = np.asarray(gamma, np.float32) / np.sqrt(
        np.asarray(moving_var, np.float32) + 1e-3
    )
    shift = np.asarray(beta, np.float32) - np.asarray(moving_mean, np.float32) * inv
    fold = bool(np.max(np.abs(shift)) == 0.0)

    # gate blocks in reference order: i,f,cc,o -> chunk0=[f;i], chunk1=[o;cc]
    perm = np.concatenate(
        [np.arange(64, 128), np.arange(0, 64), np.arange(192, 256), np.arange(128, 192)]
    )
    wx_e = kernel[:, :, :, perm]
    wr_e = rec_kernel[:, :, :, perm]
    if fold:
        # state becomes h' = h * inv  (== BN output y); compensate h-conv input
        wr_e = wr_e / inv[None, None, :, None]
    b_p = bias[perm]
    bv0 = 0.5 + 0.2 * b_p[0:128]
    bv1 = np.concatenate([0.5 + 0.2 * b_p[128:192], b_p[192:256]])
    bvec = np.stack([bv0, bv1], axis=1).astype(np.float32)
    ivv = np.stack(
        [inv if fold else np.ones(COUT, np.float32), inv, shift], axis=1
    ).astype(np.float32)
    smat = np.concatenate([np.eye(COUT), np.eye(COUT)], axis=0).astype(np.float16)

    def stack_x(w):  # [3,3,32,256] -> [128, 3, 2, 128] slot stacks
        S = np.zeros((128, 3, 2, 128), np.float32)

        def put(slot, band, ky, kx):
            blk = w[ky, kx]  # [32, 256]
            for m in (0, 1):
                S[band * 32:(band + 1) * 32, slot, m, :] = blk[:, m * 128:(m + 1) * 128]

        put(0, 0, 0, 0); put(0, 1, 0, 1); put(0, 2, 1, 0); put(0, 3, 1, 1)
        put(1, 1, 1, 2); put(1, 2, 2, 1); put(1, 3, 2, 2)   # band 0 = zeroed dup
        put(2, 0, 0, 2); put(2, 1, 2, 0)
        return S.astype(np.float16)

    def stack_h(w):  # [3,3,64,256] -> [128, 5, 2, 128] slot stacks
        S = np.zeros((128, 5, 2, 128), np.float32)

        def put(slot, half, ky, kx):
            blk = w[ky, kx]  # [64, 256]
            for m in (0, 1):
                S[half * 64:(half + 1) * 64, slot, m, :] = blk[:, m * 128:(m + 1) * 128]

        for kx in range(3):
            put(kx, 0, 0, kx); put(kx, 1, 1, kx)
        put(3, 0, 2, 0); put(3, 1, 2, 1)
        put(4, 0, 2, 2)
        return S.astype(np.float16)

    in_maps = []
    for core in range(NCORES):
        b, s = core // 2, core % 2
        xs = x[b] if s == 0 else x[b, :, ::-1]
        wx_s = wx_e if s == 0 else wx_e[::-1]
        wr_s = wr_e if s == 0 else wr_e[::-1]
        xf = np.zeros((T, CIN, FR, FW), np.float16)
        xf[:, :, 1:49, 1: W + 1] = xs[:, 0:48].transpose(0, 3, 1, 2)
        in_maps.append(
            dict(
                xin=xf,
                wxs=stack_x(wx_s),
                wrs=stack_h(wr_s),
                bvec=bvec,
                invv=ivv,
                smat=smat,
            )
        )
    return in_maps, fold


def assemble(results):
    y = np.zeros((B, T, H, W, COUT), np.float32)
    for core in range(NCORES):
        b, s = core // 2, core % 2
        blk = results[core]["yout"].transpose(0, 2, 3, 1)  # [T, 32, W, C]
        if s == 0:
            y[b, :, 0:32] = blk
        else:
            y[b, :, 32:64] = blk[:, ::-1]
    return y


_NC_CACHE: dict = {}


def get_nc(needs_affine: bool) -> bass.Bass:
    if needs_affine not in _NC_CACHE:
        _NC_CACHE[needs_affine] = _build_nc(needs_affine)
    return _NC_CACHE[needs_affine]


def kernel(**inputs) -> np.ndarray:
    in_maps, fold = prepare(**inputs)
    nc = get_nc(not fold)
    res = bass_utils.run_bass_kernel_spmd(nc, in_maps, core_ids=list(range(NCORES)))
    return assemble(res.results)
